# revision 1
# baseline (speedup 1.0000x reference)
"""Trainium2 Bass kernel for nn_CMF_Block (cross-modal fusion block).

Reference computation (per batch b):
    q = gconv1x1(rgb, w_q, b_q)   # [c, n]   c=256, n=h*w=4096, groups=4
    k = gconv1x1(ir,  w_k, b_k)
    v = gconv1x1(ir,  w_v, b_v)
    attn = softmax(q^T k * c^-0.5, axis=-1)      # [n, n]
    z = v @ attn^T                                # [c, n]
    y = w4 @ z + b4 ; y = BN(y) ; out = rgb + mish(y)

Sharding: 8 cores = 4 batches x 2 query-halves. Each core gets the full
ir slab [256, 4096] plus its rgb query-half [256, 2048] and produces the
matching disjoint output slice [256, 2048]. No collectives.

Per-core dataflow (all layouts transpose-free):
  - gconvs run in float32r (fp22 PE mode, full speed at N>=256) straight
    from the fp32 staged inputs -- no input casts.
  - q, k are written as fp8e4 [128, 2, n] (c-chunk interleaved); the
    attention-scale 1/16 is split sqrt-wise into w_q and w_k so q/k land
    in fp8's comfortable range.
  - v is computed directly transposed (lhsT = ir j-tile), vT [j, c] bf16
    with a memset ones column at col 256.
  - Flash-style fused attention: per 512-query i-group, stream 16 pairs
    of j-tiles: one DoubleRow fp8 matmul per j-tile (K=256 in one shot)
    into a [128, 1024] psum pair, one Exp per pair (ScalarE), then 8
    accumulating z matmuls one pair behind (PE never waits on ACT).
    z psum col 256 accumulates the softmax denominator for free.
  - z normalization by per-partition reciprocal, PE-transpose back to
    z[c, i] (transpose psum shares the z accumulator bank slots).
  - y = w4' @ z with BN folded into w4'/b4' and b_v folded via
    b4'' = b4' + w4' @ b_v (exact: softmax rows sum to 1). mish via
    u = e^y: tanh(softplus(y)) = 1 - 2/((u+1)^2+1) (Exp+Square on ACT,
    reciprocal on DVE; Mish LUT can't share the ACT table set with Exp).
  - Tensors are split (k/ir halves, q/z per i-group) so the attention
    starts before all of phase 2 finishes and phase 5 overlaps the
    attention tail.
"""

import sys

sys.path.insert(0, "/opt/trn_rl_repo")

import numpy as np
import ml_dtypes

import concourse.bass as bass
import concourse.tile as tile
from concourse import bacc
from concourse import mybir
from concourse.bass_utils import run_bass_kernel_spmd
from concourse.masks import make_identity

F32 = mybir.dt.float32
F32R = mybir.dt.float32r
BF16 = mybir.dt.bfloat16
FP8 = mybir.dt.float8e4
AF = mybir.ActivationFunctionType
DR = mybir.MatmulPerfMode.DoubleRow

BS, C, H, W = 4, 256, 64, 64
N = H * W              # 4096
G, CG = 4, 64
NH = N // 2            # 2048 query positions per core
NCORES = 8
SCALE = C ** -0.5      # 1/16

JT = N // 128          # 32 key tiles
IT = NH // 128         # 16 query tiles
IG = 4                 # i-groups of 512 queries
ITG = IT // IG


def build_program():
    nc = bacc.Bacc("TRN2", target_bir_lowering=False, debug=False,
                   enable_asserts=False)

    x_rgb = nc.dram_tensor("x_rgb", [C, NH], F32R, kind="ExternalInput").ap()
    x_ir = nc.dram_tensor("x_ir", [C, N], F32R, kind="ExternalInput").ap()
    wq_bd = nc.dram_tensor("wq_bd", [2, 128, 128], F32R, kind="ExternalInput").ap()
    wk_bd = nc.dram_tensor("wk_bd", [2, 128, 128], F32R, kind="ExternalInput").ap()
    wv_r = nc.dram_tensor("wv_r", [2, 128, 256], F32R, kind="ExternalInput").ap()
    w4t = nc.dram_tensor("w4t", [2, 2, 128, 128], BF16, kind="ExternalInput").ap()
    bq = nc.dram_tensor("bq", [128, 2], F32, kind="ExternalInput").ap()
    bk = nc.dram_tensor("bk", [128, 2], F32, kind="ExternalInput").ap()
    b4 = nc.dram_tensor("b4", [128, 2], F32, kind="ExternalInput").ap()
    out = nc.dram_tensor("out", [C, NH], F32, kind="ExternalOutput").ap()

    with tile.TileContext(nc) as tc:
        with tc.tile_pool(name="persist", bufs=1) as persist:
            # split tensors for fine-grained dependencies
            qsg = [persist.tile([128, 2, 512], FP8, tag=f"qsg{g}",
                                name=f"qsg{g}") for g in range(IG)]
            ksh = [persist.tile([128, 2, 2048], FP8, tag=f"ksh{h}",
                                name=f"ksh{h}") for h in range(2)]
            vTh = [persist.tile([128, 16, 257], BF16, tag=f"vTh{h}",
                                name=f"vTh{h}") for h in range(2)]
            zsg = [[persist.tile([128, 512], BF16, tag=f"zsg{ch}_{g}",
                                 name=f"zsg{ch}_{g}") for g in range(IG)]
                   for ch in range(2)]
            rgbf = [persist.tile([128, NH], F32R, tag=f"rgbf{ch}",
                                 name=f"rgbf{ch}") for ch in range(2)]
            wq_sb = persist.tile([128, 2, 128], F32R, tag="wq_sb", name="wq_sb")
            wk_sb = persist.tile([128, 2, 128], F32R, tag="wk_sb", name="wk_sb")
            wv_sb = persist.tile([128, 2, 256], F32R, tag="wv_sb", name="wv_sb")
            w4_sb = persist.tile([128, 2, 2, 128], BF16, tag="w4_sb", name="w4_sb")
            bq_sb = persist.tile([128, 2], F32, tag="bq_sb", name="bq_sb")
            bk_sb = persist.tile([128, 2], F32, tag="bk_sb", name="bk_sb")
            b4_sb = persist.tile([128, 2], F32, tag="b4_sb", name="b4_sb")
            ident = persist.tile([128, 128], BF16, tag="ident", name="ident")

            for ch in range(2):
                nc.sync.dma_start(wq_sb[:, ch], wq_bd[ch])
                nc.sync.dma_start(wk_sb[:, ch], wk_bd[ch])
                nc.sync.dma_start(wv_sb[:, ch], wv_r[ch])
                for oh in range(2):
                    nc.sync.dma_start(w4_sb[:, ch, oh], w4t[ch, oh])
            nc.sync.dma_start(bq_sb[:], bq)
            nc.sync.dma_start(bk_sb[:], bk)
            nc.sync.dma_start(b4_sb[:], b4)
            make_identity(nc, ident[:])
            for h in range(2):
                nc.vector.memset(vTh[h][:, :, 256], 1.0)

            # ---------------- Phase 1+2: load ir + gconvs (float32r) ------
            with (
                tc.tile_pool(name="irp", bufs=1) as irp,
                tc.tile_pool(name="qk_psum", bufs=4, space="PSUM") as qk_psum,
                tc.tile_pool(name="vt_psum", bufs=4, space="PSUM") as vt_psum,
            ):
                irfh = [[irp.tile([128, 2048], F32R, tag=f"irf{ch}_{h}",
                                  name=f"irf{ch}_{h}") for h in range(2)]
                        for ch in range(2)]
                for ch in range(2):
                    nc.sync.dma_start(irfh[ch][0][:],
                                      x_ir[ch * 128:(ch + 1) * 128, 0:2048])
                for ch in range(2):
                    nc.sync.dma_start(rgbf[ch][:],
                                      x_rgb[ch * 128:(ch + 1) * 128, :])
                for ch in range(2):
                    nc.sync.dma_start(irfh[ch][1][:],
                                      x_ir[ch * 128:(ch + 1) * 128, 2048:4096])

                def kconv(h):
                    # k gconv n-half h -> fp8 ksh[h]
                    for ch in range(2):
                        for q4 in range(4):
                            ps = qk_psum.tile([128, 512], F32, tag="qk",
                                              name="qk")
                            nsl = slice(q4 * 512, (q4 + 1) * 512)
                            nc.tensor.matmul(ps[:], wk_sb[:, ch],
                                             irfh[ch][h][:, nsl],
                                             start=True, stop=True)
                            dst = ksh[h][:, ch, q4 * 512:(q4 + 1) * 512]
                            if q4 % 2 == 0:
                                nc.vector.tensor_scalar_add(
                                    dst, ps[:], bk_sb[:, ch:ch + 1])
                            else:
                                nc.scalar.activation(
                                    dst, ps[:], AF.Identity,
                                    bias=bk_sb[:, ch:ch + 1])

                def vconv(h):
                    # vT for j-tiles of n-half h
                    for j in range(16):
                        ps = vt_psum.tile([128, 256], F32, tag="vt", name="vt")
                        jsl = slice(j * 128, (j + 1) * 128)
                        for ch in range(2):
                            nc.tensor.matmul(ps[:], irfh[ch][h][:, jsl],
                                             wv_sb[:, ch],
                                             start=(ch == 0), stop=(ch == 1))
                        if j % 2 == 0:
                            nc.vector.tensor_copy(vTh[h][:, j, 0:256], ps[:])
                        else:
                            nc.scalar.copy(vTh[h][:, j, 0:256], ps[:])

                kconv(0)
                # q gconv per i-group -> fp8 qsg[g]
                for g in range(IG):
                    gsl = slice(g * 512, (g + 1) * 512)
                    for ch in range(2):
                        ps = qk_psum.tile([128, 512], F32, tag="qk", name="qk")
                        nc.tensor.matmul(ps[:], wq_sb[:, ch],
                                         rgbf[ch][:, gsl],
                                         start=True, stop=True)
                        if (g + ch) % 2 == 0:
                            nc.vector.tensor_scalar_add(
                                qsg[g][:, ch, :], ps[:], bq_sb[:, ch:ch + 1])
                        else:
                            nc.scalar.activation(
                                qsg[g][:, ch, :], ps[:], AF.Identity,
                                bias=bq_sb[:, ch:ch + 1])
                kconv(1)
                vconv(0)
                vconv(1)

            # ---- Fused flash attention + overlapped phase 5 --------------
            PAIRS = JT // 2
            with (
                tc.tile_pool(name="pexp", bufs=6) as pexp,
                tc.tile_pool(name="znorm", bufs=8) as znorm,
                tc.tile_pool(name="fin", bufs=3) as fin,
                tc.tile_pool(name="s_psum", bufs=2, space="PSUM") as s_psum,
                tc.tile_pool(name="z_psum", bufs=4, space="PSUM") as z_psum,
            ):
                def phase5(g):
                    # y chunk for i-group g; y psum shares z_psum slots
                    nsl = slice(g * 512, (g + 1) * 512)
                    for oh in range(2):
                        ps = z_psum.tile([128, 512], F32, tag="zT", name="y")
                        for ch in range(2):
                            nc.tensor.matmul(ps[:], w4_sb[:, ch, oh],
                                             zsg[ch][g][:],
                                             start=(ch == 0), stop=(ch == 1))
                        # mish(y) = y*tanh(softplus(y)); u = e^y:
                        # tanh(ln(1+u)) = 1 - 2/((u+1)^2 + 1)
                        bias = b4_sb[:, oh:oh + 1]
                        u = fin.tile([128, 512], F32, tag="u", name="u")
                        nc.scalar.activation(u[:], ps[:], AF.Exp, bias=bias)
                        yb = fin.tile([128, 512], F32, tag="yb", name="yb")
                        nc.vector.tensor_scalar_add(yb[:], ps[:], bias)
                        u1 = fin.tile([128, 512], F32, tag="u1", name="u1")
                        nc.scalar.activation(u1[:], u[:], AF.Square, bias=1.0)
                        d = fin.tile([128, 512], F32, tag="d", name="d")
                        nc.vector.tensor_scalar_add(d[:], u1[:], 1.0)
                        r = fin.tile([128, 512], F32, tag="r", name="r")
                        nc.vector.reciprocal(r[:], d[:])
                        t = fin.tile([128, 512], F32, tag="t", name="t")
                        nc.vector.tensor_scalar(t[:], r[:], -2.0, 1.0,
                                                mybir.AluOpType.mult,
                                                mybir.AluOpType.add)
                        m = fin.tile([128, 512], F32, tag="mish", name="mish")
                        nc.vector.tensor_mul(m[:], yb[:], t[:])
                        o = fin.tile([128, 512], F32, tag="osb", name="osb")
                        nc.vector.tensor_add(o[:], m[:],
                                             rgbf[oh][:, nsl].bitcast(F32))
                        nc.sync.dma_start(out[oh * 128:(oh + 1) * 128, nsl],
                                          o[:])

                for ig in range(IG):
                    zps = [z_psum.tile([128, 257], F32, tag="zT",
                                       name=f"zT{t}") for t in range(ITG)]
                    pending = []

                    def flush(pair, zps=zps):
                        ppt, pr = pair
                        for hh in range(2):
                            jt = 2 * pr + hh
                            for t in range(ITG):
                                nc.tensor.matmul(
                                    zps[t][:],
                                    ppt[:, hh * 512 + t * 128:
                                        hh * 512 + (t + 1) * 128],
                                    vTh[jt // 16][:, jt % 16],
                                    start=(jt == 0), stop=(jt == JT - 1))

                    for pr in range(PAIRS):
                        ps = s_psum.tile([128, 1024], F32, tag="sT", name="sT")
                        for hh in range(2):
                            jt = 2 * pr + hh
                            jsl = slice((jt % 16) * 128, (jt % 16 + 1) * 128)
                            nc.tensor.matmul(ps[:, hh * 512:(hh + 1) * 512],
                                             ksh[jt // 16][:, :, jsl],
                                             qsg[ig][:],
                                             perf_mode=DR, start=True,
                                             stop=True)
                        if len(pending) > 2:
                            flush(pending.pop(0))
                        pt = pexp.tile([128, 1024], BF16, tag="pt", name="pt")
                        nc.scalar.activation(pt[:], ps[:], AF.Exp)
                        pending.append((pt, pr))
                    for pair in pending:
                        flush(pair)

                    # normalize + transpose back to z[c, i] (batched so
                    # the per-i-tile chains pipeline across engines)
                    rinvs, zns = [], []
                    for t in range(ITG):
                        rinv = znorm.tile([128, 1], F32, tag="rinv",
                                          name="rinv")
                        nc.vector.reciprocal(rinv[:], zps[t][:, 256:257])
                        rinvs.append(rinv)
                    for t in range(ITG):
                        zn = znorm.tile([128, 256], BF16, tag="zn", name="zn")
                        nc.vector.tensor_scalar_mul(zn[:], zps[t][:, 0:256],
                                                    rinvs[t][:])
                        zns.append(zn)
                    for t in range(ITG):
                        for ch in range(2):
                            tp = z_psum.tile([128, 128], BF16, tag="zT",
                                             name="tp")
                            nc.tensor.transpose(
                                tp[:], zns[t][:, ch * 128:(ch + 1) * 128],
                                ident[:])
                            nc.vector.tensor_copy(
                                zsg[ch][ig][:, t * 128:(t + 1) * 128], tp[:])
                    phase5(ig)

    nc.finalize()
    return nc


def _blockdiag_T(w, g0, g1):
    """lhsT chunk: [[w[g0].T, 0], [0, w[g1].T]] as [128, 128]."""
    m = np.zeros((128, 128), dtype=np.float64)
    m[:64, :64] = w[g0].T
    m[64:, 64:] = w[g1].T
    return m


def prep_inputs(rgb, ir, w_q, b_q, w_k, b_k, w_v, b_v, w4, b4,
                gamma, beta, rmean, rvar):
    """Host-side prep: fold scale/BN/b_v, pack block-diagonal weights."""
    f64 = np.float64
    w_q, b_q = f64(np.asarray(w_q)), f64(np.asarray(b_q))
    w_k, b_k = f64(np.asarray(w_k)), f64(np.asarray(b_k))
    w_v, b_v = f64(np.asarray(w_v)), f64(np.asarray(b_v))
    w4, b4 = f64(np.asarray(w4)), f64(np.asarray(b4))
    gamma, beta = f64(np.asarray(gamma)), f64(np.asarray(beta))
    rmean, rvar = f64(np.asarray(rmean)), f64(np.asarray(rvar))

    inv = gamma / np.sqrt(rvar + 1e-5)
    w4f = w4 * inv[:, None]                      # BN folded into w4
    b4f = b4 * inv + beta - rmean * inv + w4f @ b_v   # b_v folded

    f32 = np.float32
    bf16 = ml_dtypes.bfloat16
    hs = np.sqrt(SCALE)  # split attention scale between q and k for fp8 range
    wq_bd = np.stack([_blockdiag_T(w_q * hs, 0, 1),
                      _blockdiag_T(w_q * hs, 2, 3)]).astype(f32)
    wk_bd = np.stack([_blockdiag_T(w_k * hs, 0, 1),
                      _blockdiag_T(w_k * hs, 2, 3)]).astype(f32)
    wv_r = np.zeros((2, 128, 256), dtype=np.float64)
    wv_r[0, :, 0:128] = _blockdiag_T(w_v, 0, 1)
    wv_r[1, :, 128:256] = _blockdiag_T(w_v, 2, 3)
    wv_r = wv_r.astype(f32)
    w4t = np.zeros((2, 2, 128, 128), dtype=np.float64)
    for ch in range(2):
        for oh in range(2):
            w4t[ch, oh] = w4f[oh * 128:(oh + 1) * 128,
                              ch * 128:(ch + 1) * 128].T
    w4t = w4t.astype(bf16)

    def cols(v):
        return np.stack([v[:128], v[128:]], axis=1).astype(np.float32)

    bq_c = cols(b_q * hs)
    bk_c = cols(b_k * hs)
    b4_c = cols(b4f)

    rgb_f = np.ascontiguousarray(np.asarray(rgb), dtype=np.float32)
    ir_f = np.ascontiguousarray(np.asarray(ir), dtype=np.float32)

    weights = dict(wq_bd=wq_bd, wk_bd=wk_bd, wv_r=wv_r, w4t=w4t,
                   bq=bq_c, bk=bk_c, b4=b4_c)
    in_maps = []
    for core in range(NCORES):
        b, half = divmod(core, 2)
        x_rgb = np.ascontiguousarray(
            rgb_f[b].reshape(C, N)[:, half * NH:(half + 1) * NH])
        x_ir = np.ascontiguousarray(ir_f[b].reshape(C, N))
        in_maps.append(dict(x_rgb=x_rgb, x_ir=x_ir, **weights))
    return in_maps


_PROGRAM = None


def _get_program():
    global _PROGRAM
    if _PROGRAM is None:
        _PROGRAM = build_program()
    return _PROGRAM


def run(inputs, trace=False, **kw):
    """Run on 8 cores; returns (full_output, BassKernelResults)."""
    nc = _get_program()
    in_maps = prep_inputs(**inputs)
    res = run_bass_kernel_spmd(nc, in_maps, list(range(NCORES)),
                               trace=trace, **kw)
    full = np.zeros((BS, C, H, W), dtype=np.float32)
    for core in range(NCORES):
        b, half = divmod(core, 2)
        full[b].reshape(C, N)[:, half * NH:(half + 1) * NH] = \
            res.results[core]["out"]
    return full, res


def kernel(**inputs) -> np.ndarray:
    out, _ = run(inputs)
    return out



# revision 2
# speedup vs baseline: 1.2207x; 1.2207x over previous
"""Trainium2 Bass kernel for nn_CMF_Block (cross-modal fusion block).

Reference computation (per batch b):
    q = gconv1x1(rgb, w_q, b_q)   # [c, n]   c=256, n=h*w=4096, groups=4
    k = gconv1x1(ir,  w_k, b_k)
    v = gconv1x1(ir,  w_v, b_v)
    attn = softmax(q^T k * c^-0.5, axis=-1)      # [n, n]
    z = v @ attn^T                                # [c, n]
    y = w4 @ z + b4 ; y = BN(y) ; out = rgb + mish(y)

Sharding: 8 cores = 4 batches x 2 query-halves. Each core gets the full
ir slab [256, 4096] plus its rgb query-half [256, 2048] and produces the
matching disjoint output slice [256, 2048]. No collectives.

v3 design -- all heavy matmuls in fp8 DoubleRow (0.5 cyc/row), minimum
ACT/DVE traffic:
  - k never exists on device: scores = ir^T q' with q' = wk^T(wq rgb+bq)
    (wqk = blockdiag(wk)^T blockdiag(wq) folded on host, still
    block-diagonal). The per-query term (q+bq).bk dropped -- it cancels
    exactly in softmax.
  - ir is DMA'd straight from DRAM to fp8 [c-chunk-pair, n] layout via
    gpsimd cast-DMA; it serves as DoubleRow lhsT for both the scores and
    the v' conv.
  - w4+BN fold into the v side: W' = w4f blockdiag(wv) (dense), so the
    attention-weighted matmul directly yields y numerators [i, o] with a
    ones column accumulating the softmax denominator. b_v and b4 fold
    into a single output bias (exact: softmax rows sum to 1).
  - exp of scores is split between ACT (true Exp, scale=1/16, fp8 out)
    and DVE (one-op Schraudolph: uint8(A*s+B) whose bytes ARE fp8e4m3 of
    exp(s/16), ~3% rms -- diluted ~300x in the output since out =
    rgb + mish(y) and the attention ripple is a tiny part of y).
  - z/y accumulate in fp8 DR over 16 pair-steps into [128,257] psum.
  - normalize by 0.125/denominator (v'-weights carry x8 for fp8 range),
    PE-transpose y back to [o, i], mish via Exp+Square (one ACT table,
    zero table swaps) with the rational tail in bf16 on DVE, final
    x + rgb add and a few elementwise steps on the otherwise-idle Pool
    (gpsimd) engine.
  - phase5 of group g is software-pipelined into group g+1's pair loop.
"""

import sys

sys.path.insert(0, "/opt/trn_rl_repo")

import numpy as np
import ml_dtypes

import concourse.bass as bass
import concourse.tile as tile
from concourse import bacc
from concourse import mybir
from concourse.bass_utils import run_bass_kernel_spmd
from concourse.masks import make_identity

F32 = mybir.dt.float32
F32R = mybir.dt.float32r
BF16 = mybir.dt.bfloat16
FP8 = mybir.dt.float8e4
U8 = mybir.dt.uint8
AF = mybir.ActivationFunctionType
DR = mybir.MatmulPerfMode.DoubleRow
ALU = mybir.AluOpType

BS, C, H, W = 4, 256, 64, 64
N = H * W              # 4096
NH = N // 2            # 2048 query positions per core
NCORES = 8
G, CG = 4, 64

JT = N // 128          # 32 key tiles
PAIRS = JT // 2        # 16 key-tile pairs (DoubleRow K=256 steps)
IG = 4                 # i-groups of 512 queries
ITG = 4                # 128-query tiles per group

SCALE = 1.0 / 16.0     # c^-0.5, applied inside exp
VSCALE = 8.0           # fp8-range headroom for W'; undone in normalize
LOG2E = 1.4426950408889634
EXP_A = LOG2E * SCALE * 8.0   # Schraudolph: byte = A*s_raw + B
EXP_B = 56.0 - 0.35

# pairs whose exp runs on DVE (Schraudolph); rest on ACT (true Exp)
DVE_PRS = frozenset((1, 3, 5, 8, 10, 12, 14))


def build_program():
    nc = bacc.Bacc("TRN2", target_bir_lowering=False, debug=False,
                   enable_asserts=False)

    x_rgb = nc.dram_tensor("x_rgb", [C, NH], F32R, kind="ExternalInput").ap()
    x_ir = nc.dram_tensor("x_ir", [C, N], F32, kind="ExternalInput").ap()
    wqk = nc.dram_tensor("wqk", [2, 128, 128], F32R, kind="ExternalInput").ap()
    bqk = nc.dram_tensor("bqk", [128, 2], F32, kind="ExternalInput").ap()
    wvp = nc.dram_tensor("wvp", [128, 2, 256], U8, kind="ExternalInput").ap()
    b4c = nc.dram_tensor("b4c", [128, 2], F32, kind="ExternalInput").ap()
    out = nc.dram_tensor("out", [C, NH], F32, kind="ExternalOutput").ap()

    with tile.TileContext(nc) as tc:
        with tc.tile_pool(name="persist", bufs=1) as persist:
            rgbf = persist.tile([128, 2, NH], F32R, tag="rgbf", name="rgbf")
            ir8 = persist.tile([128, 2, N], FP8, tag="ir8", name="ir8")
            qsg = [persist.tile([128, 2, 512], FP8, tag=f"qsg{g}",
                                name=f"qsg{g}") for g in range(IG)]
            vTp = persist.tile([128, PAIRS, 2, 257], FP8, tag="vTp",
                               name="vTp")
            znb = [persist.tile([128, ITG, 256], BF16, tag=f"znb{g}",
                                name=f"znb{g}") for g in range(IG)]
            rvb = [persist.tile([128, ITG], F32, tag=f"rvb{g}",
                                name=f"rvb{g}") for g in range(IG)]
            wqk_sb = persist.tile([128, 2, 128], F32R, tag="wqk_sb",
                                  name="wqk_sb")
            bqk_sb = persist.tile([128, 2], F32, tag="bqk_sb", name="bqk_sb")
            wv_sb = persist.tile([128, 2, 256], U8, tag="wv_sb", name="wv_sb")
            b4_sb = persist.tile([128, 2], F32, tag="b4_sb", name="b4_sb")
            ident = persist.tile([128, 128], BF16, tag="ident", name="ident")

            for ch in range(2):
                nc.sync.dma_start(wqk_sb[:, ch], wqk[ch])
            nc.sync.dma_start(bqk_sb[:], bqk)
            nc.sync.dma_start(wv_sb[:], wvp)
            nc.sync.dma_start(b4_sb[:], b4c)
            make_identity(nc, ident[:])
            nc.vector.memset(vTp[:, :, :, 256], 1.0)

            # ir: DRAM f32 -> SBUF fp8 pair layout via gpsimd cast-DMA,
            # quarter-granular so consumers start early.  rgb via HWDGE.
            for quarter in range(4):
                nsl = slice(quarter * 1024, (quarter + 1) * 1024)
                for ch in range(2):
                    nc.gpsimd.dma_start(ir8[:, ch, nsl],
                                        x_ir[ch * 128:(ch + 1) * 128, nsl])
            for ch in range(2):
                nc.sync.dma_start(rgbf[:, ch], x_rgb[ch * 128:(ch + 1) * 128])

            wv8 = wv_sb[:].bitcast(FP8)

            # ---------------- Phase 2: q' conv + v' conv ------------------
            with (
                tc.tile_pool(name="qp_psum", bufs=2, space="PSUM") as qp,
                tc.tile_pool(name="vp_psum", bufs=2, space="PSUM") as vp,
            ):
                def vconv(pr):
                    # one DoubleRow matmul per j-tile, pair-packed psum
                    ps = vp.tile([128, 2, 256], F32, tag="vps", name="vps")
                    for m in range(2):
                        jt = 2 * pr + m
                        jsl = slice(jt * 128, (jt + 1) * 128)
                        nc.tensor.matmul(ps[:, m], ir8[:, :, jsl], wv8,
                                         perf_mode=DR, start=True, stop=True)
                    dst = vTp[:, pr, :, 0:256]
                    if pr % 2 == 0:
                        nc.scalar.copy(dst, ps[:])
                    else:
                        nc.vector.tensor_copy(dst, ps[:])

                def qconv(g):
                    gsl = slice(g * 512, (g + 1) * 512)
                    for ch in range(2):
                        ps = qp.tile([128, 512], F32, tag="qps", name="qps")
                        nc.tensor.matmul(ps[:], wqk_sb[:, ch],
                                         rgbf[:, ch, gsl],
                                         start=True, stop=True)
                        if (g + ch) % 2 == 0:
                            nc.scalar.activation(qsg[g][:, ch], ps[:],
                                                 AF.Identity,
                                                 bias=bqk_sb[:, ch:ch + 1])
                        else:
                            nc.vector.tensor_scalar_add(
                                qsg[g][:, ch], ps[:], bqk_sb[:, ch:ch + 1])

                for pr in range(4):
                    vconv(pr)
                qconv(0)
                qconv(1)
                for pr in range(4, 10):
                    vconv(pr)
                qconv(2)
                qconv(3)
                for pr in range(10, PAIRS):
                    vconv(pr)

            # ---- Phase 3+5: fused attention, phase5 pipelined ------------
            with (
                tc.tile_pool(name="pexp", bufs=6) as pexp,
                tc.tile_pool(name="ph5", bufs=3) as ph5,
                tc.tile_pool(name="s_psum", bufs=2, space="PSUM") as sp,
                tc.tile_pool(name="z_psum", bufs=4, space="PSUM") as zp,
            ):
                def phase5_steps(g, zps):
                    """Deferred phase5 for group g as a list of closures."""
                    gsl = slice(g * 512, (g + 1) * 512)
                    state = {}

                    def s_norm():
                        rv = rvb[g]
                        for t in range(ITG):
                            nc.vector.reciprocal(rv[:, t:t + 1],
                                                 zps[t][:, 256:257])
                        nc.vector.tensor_scalar_mul(rv[:], rv[:], 1.0 / VSCALE)
                        for t in range(ITG):
                            if t % 2 == 0:
                                nc.scalar.activation(znb[g][:, t],
                                                     zps[t][:, 0:256],
                                                     AF.Copy,
                                                     scale=rv[:, t:t + 1])
                            else:
                                nc.vector.tensor_scalar_mul(
                                    znb[g][:, t], zps[t][:, 0:256],
                                    rv[:, t:t + 1])

                    def s_transpose():
                        yT = zp.tile([128, 2, 512], BF16, tag="zt", name="yT")
                        for t in range(ITG):
                            for ch in range(2):
                                nc.tensor.transpose(
                                    yT[:, ch, t * 128:(t + 1) * 128],
                                    znb[g][:, t, ch * 128:(ch + 1) * 128],
                                    ident[:])
                        state["yT"] = yT

                    def s_act():
                        yT = state["yT"]
                        u = ph5.tile([128, 2, 512], BF16, tag="u", name="u")
                        yb = ph5.tile([128, 2, 512], BF16, tag="yb",
                                      name="yb")
                        for ch in range(2):
                            nc.scalar.activation(u[:, ch], yT[:, ch], AF.Exp,
                                                 bias=b4_sb[:, ch:ch + 1])
                            nc.vector.tensor_scalar_add(
                                yb[:, ch], yT[:, ch], b4_sb[:, ch:ch + 1])
                        s = ph5.tile([128, 2, 512], BF16, tag="s", name="s")
                        nc.scalar.activation(s[:], u[:], AF.Square, bias=1.0)
                        state["yb"], state["s"] = yb, s

                    def s_tail():
                        yb, s = state["yb"], state["s"]
                        d = ph5.tile([128, 2, 512], BF16, tag="d", name="d")
                        nc.gpsimd.tensor_scalar_add(d[:], s[:], 1.0)
                        r = ph5.tile([128, 2, 512], BF16, tag="r", name="r")
                        with nc.allow_low_precision(
                                reason="bf16 mish tail; 2e-2 gate"):
                            nc.vector.reciprocal(r[:], d[:])
                        tt = ph5.tile([128, 2, 512], BF16, tag="tt",
                                      name="tt")
                        nc.vector.tensor_scalar(tt[:], r[:], -2.0, 1.0,
                                                ALU.mult, ALU.add)
                        m = ph5.tile([128, 2, 512], BF16, tag="m", name="m")
                        nc.gpsimd.tensor_mul(m[:], yb[:], tt[:])
                        o = ph5.tile([128, 2, 512], F32, tag="o", name="o")
                        nc.gpsimd.tensor_add(o[:], m[:],
                                             rgbf[:, :, gsl].bitcast(F32))
                        for ch in range(2):
                            nc.sync.dma_start(
                                out[ch * 128:(ch + 1) * 128, gsl], o[:, ch])

                    return [s_norm, s_transpose, s_act, s_tail]

                # checkpoints in the next group's pair loop where the
                # previous group's phase5 steps are emitted
                CKPT = {1: 0, 4: 1, 7: 2, 10: 3}

                prev_steps = None
                for g in range(IG):
                    zps = [zp.tile([128, 257], F32, tag="zt", name=f"z{t}")
                           for t in range(ITG)]
                    pending = []

                    def flush(pair, zps=zps):
                        pt, pr = pair
                        for t in range(ITG):
                            nc.tensor.matmul(
                                zps[t][:], pt[:, :, t * 128:(t + 1) * 128],
                                vTp[:, pr], perf_mode=DR,
                                start=(pr == 0), stop=(pr == PAIRS - 1))

                    for pr in range(PAIRS):
                        if prev_steps is not None and pr in CKPT:
                            prev_steps[CKPT[pr]]()
                        ps = sp.tile([128, 2, 512], F32, tag="sT", name="sT")
                        for m in range(2):
                            jt = 2 * pr + m
                            jsl = slice(jt * 128, (jt + 1) * 128)
                            nc.tensor.matmul(ps[:, m], ir8[:, :, jsl],
                                             qsg[g][:], perf_mode=DR,
                                             start=True, stop=True)
                        if len(pending) > 1:
                            flush(pending.pop(0))
                        pt = pexp.tile([128, 2, 512], FP8, tag="pt",
                                       name="pt")
                        if pr in DVE_PRS:
                            nc.vector.tensor_scalar(pt[:].bitcast(U8), ps[:],
                                                    EXP_A, EXP_B,
                                                    ALU.mult, ALU.add)
                        else:
                            nc.scalar.activation(pt[:], ps[:], AF.Exp,
                                                 scale=SCALE)
                        pending.append((pt, pr))
                    for pair in pending:
                        flush(pair)
                    prev_steps = phase5_steps(g, zps)

                for step in prev_steps:
                    step()

    nc.finalize()
    return nc


def _blockdiag_T(w, g0, g1):
    """lhsT chunk: [[w[g0].T, 0], [0, w[g1].T]] as [128, 128]."""
    m = np.zeros((128, 128), dtype=np.float64)
    m[:64, :64] = w[g0].T
    m[64:, 64:] = w[g1].T
    return m


def prep_inputs(rgb, ir, w_q, b_q, w_k, b_k, w_v, b_v, w4, b4,
                gamma, beta, rmean, rvar):
    """Host-side weight folding (weights-only; no data-dependent work)."""
    f64 = np.float64
    w_q, b_q = f64(np.asarray(w_q)), f64(np.asarray(b_q))
    w_k = f64(np.asarray(w_k))
    w_v, b_v = f64(np.asarray(w_v)), f64(np.asarray(b_v))
    w4, b4 = f64(np.asarray(w4)), f64(np.asarray(b4))
    gamma, beta = f64(np.asarray(gamma)), f64(np.asarray(beta))
    rmean, rvar = f64(np.asarray(rmean)), f64(np.asarray(rvar))

    inv = gamma / np.sqrt(rvar + 1e-5)
    w4f = w4 * inv[:, None]                      # BN folded into w4
    b4pp = b4 * inv + beta - rmean * inv + w4f @ b_v  # b_v folded (exact)

    # q'-side fold: scores = ir^T q',  q' = wqk rgb + bqk (block-diagonal)
    wqk = np.stack([w_k[g].T @ w_q[g] for g in range(G)])
    bqk = np.concatenate([w_k[g].T @ b_q[g * CG:(g + 1) * CG]
                          for g in range(G)])

    # v'-side fold: W' = w4f blockdiag(w_v), fp8-packed rhs [p, chunk, o]
    bd = np.zeros((C, C))
    for g in range(G):
        bd[g * CG:(g + 1) * CG, g * CG:(g + 1) * CG] = w_v[g]
    Wp = (w4f @ bd) * VSCALE
    WpT = Wp.T                                  # [a, o]
    wvp = np.ascontiguousarray(
        WpT.reshape(2, 128, C).transpose(1, 0, 2))
    wvp_u8 = wvp.astype(np.float32).astype(
        ml_dtypes.float8_e4m3).view(np.uint8)

    f32 = np.float32
    wqk_bd = np.stack([_blockdiag_T(wqk, 0, 1),
                       _blockdiag_T(wqk, 2, 3)]).astype(f32)

    def cols(v):
        return np.stack([v[:128], v[128:]], axis=1).astype(f32)

    bqk_c = cols(bqk)
    b4_c = cols(b4pp)

    rgb_f = np.ascontiguousarray(np.asarray(rgb), dtype=f32)
    ir_f = np.ascontiguousarray(np.asarray(ir), dtype=f32)

    weights = dict(wqk=wqk_bd, bqk=bqk_c, wvp=wvp_u8, b4c=b4_c)
    in_maps = []
    for core in range(NCORES):
        b, half = divmod(core, 2)
        x_rgb = np.ascontiguousarray(
            rgb_f[b].reshape(C, N)[:, half * NH:(half + 1) * NH])
        x_ir = np.ascontiguousarray(ir_f[b].reshape(C, N))
        in_maps.append(dict(x_rgb=x_rgb, x_ir=x_ir, **weights))
    return in_maps


_PROGRAM = None


def _get_program():
    global _PROGRAM
    if _PROGRAM is None:
        _PROGRAM = build_program()
    return _PROGRAM


def run(inputs, trace=False, **kw):
    """Run on 8 cores; returns (full_output, BassKernelResults)."""
    nc = _get_program()
    in_maps = prep_inputs(**inputs)
    res = run_bass_kernel_spmd(nc, in_maps, list(range(NCORES)),
                               trace=trace, **kw)
    full = np.zeros((BS, C, H, W), dtype=np.float32)
    for core in range(NCORES):
        b, half = divmod(core, 2)
        full[b].reshape(C, N)[:, half * NH:(half + 1) * NH] = \
            res.results[core]["out"]
    return full, res


def kernel(**inputs) -> np.ndarray:
    out, _ = run(inputs)
    return out


# revision 7
# speedup vs baseline: 1.3666x; 1.1195x over previous
"""Trainium2 Bass kernel for nn_CMF_Block (cross-modal fusion block).

Reference computation (per batch b):
    q = gconv1x1(rgb, w_q, b_q)   # [c, n]   c=256, n=h*w=4096, groups=4
    k = gconv1x1(ir,  w_k, b_k)
    v = gconv1x1(ir,  w_v, b_v)
    attn = softmax(q^T k * c^-0.5, axis=-1)      # [n, n]
    z = v @ attn^T                                # [c, n]
    y = w4 @ z + b4 ; y = BN(y) ; out = rgb + mish(y)

Sharding: 8 cores = 4 batches x 2 query-halves. Each core gets the full
ir slab [256, 4096] plus its rgb query-half [256, 2048] and produces the
matching disjoint output slice [256, 2048]. No collectives.

v3 design -- all heavy matmuls in fp8 DoubleRow (0.5 cyc/row), minimum
ACT/DVE traffic:
  - k never exists on device: scores = ir^T q' with q' = wk^T(wq rgb+bq)
    (wqk = blockdiag(wk)^T blockdiag(wq) folded on host, still
    block-diagonal). The per-query term (q+bq).bk dropped -- it cancels
    exactly in softmax.
  - ir is DMA'd straight from DRAM to fp8 [c-chunk-pair, n] layout via
    gpsimd cast-DMA; it serves as DoubleRow lhsT for both the scores and
    the v' conv.
  - w4+BN fold into the v side: W' = w4f blockdiag(wv) (dense), so the
    attention-weighted matmul directly yields y numerators [i, o] with a
    ones column accumulating the softmax denominator. b_v and b4 fold
    into a single output bias (exact: softmax rows sum to 1).
  - exp of scores is split between ACT (true Exp, scale=1/16, fp8 out)
    and DVE (one-op Schraudolph: uint8(A*s+B) whose bytes ARE fp8e4m3 of
    exp(s/16), ~3% rms -- diluted ~300x in the output since out =
    rgb + mish(y) and the attention ripple is a tiny part of y).
  - z/y accumulate in fp8 DR over 16 pair-steps into [128,257] psum.
  - normalize by 0.125/denominator (v'-weights carry x8 for fp8 range),
    PE-transpose y back to [o, i], mish via Exp+Square (one ACT table,
    zero table swaps) with the rational tail in bf16 on DVE, final
    x + rgb add and a few elementwise steps on the otherwise-idle Pool
    (gpsimd) engine.
  - phase5 of group g is software-pipelined into group g+1's pair loop.
"""

import sys

sys.path.insert(0, "/opt/trn_rl_repo")

import numpy as np
import ml_dtypes

import concourse.bass as bass
import concourse.tile as tile
from concourse import bacc
from concourse import mybir
from concourse.bass_utils import run_bass_kernel_spmd
from concourse.masks import make_identity

F32 = mybir.dt.float32
F32R = mybir.dt.float32r
BF16 = mybir.dt.bfloat16
FP8 = mybir.dt.float8e4
U8 = mybir.dt.uint8
AF = mybir.ActivationFunctionType
DR = mybir.MatmulPerfMode.DoubleRow
ALU = mybir.AluOpType

BS, C, H, W = 4, 256, 64, 64
N = H * W              # 4096
NH = N // 2            # 2048 query positions per core
NCORES = 8
G, CG = 4, 64

JT = N // 128          # 32 key tiles
PAIRS = JT // 2        # 16 key-tile pairs (DoubleRow K=256 steps)
IG = 4                 # i-groups of 512 queries
ITG = 4                # 128-query tiles per group

SCALE = 1.0 / 16.0     # c^-0.5, applied inside exp
VSCALE = 8.0           # fp8-range headroom for W'; undone in normalize
LOG2E = 1.4426950408889634
EXP_A = LOG2E * SCALE * 8.0   # Schraudolph: byte = A*s_raw + B
EXP_B = 56.0 - 0.35

# pairs whose exp runs on DVE (Schraudolph); rest on ACT (true Exp)
DVE_PRS = frozenset((1, 3, 5, 7, 9, 11, 13))
ZLAG = 4               # z-matmul flush lag (exp pipeline depth)


def build_program():
    nc = bacc.Bacc("TRN2", target_bir_lowering=False, debug=False,
                   enable_asserts=False)

    x_rgb = nc.dram_tensor("x_rgb", [C, NH], F32R, kind="ExternalInput").ap()
    x_ir = nc.dram_tensor("x_ir", [C, N], F32, kind="ExternalInput").ap()
    wqk = nc.dram_tensor("wqk", [2, 128, 128], F32R, kind="ExternalInput").ap()
    bqk = nc.dram_tensor("bqk", [128, 2], F32, kind="ExternalInput").ap()
    wvp = nc.dram_tensor("wvp", [128, 2, 256], U8, kind="ExternalInput").ap()
    b4c = nc.dram_tensor("b4c", [128, 2], F32, kind="ExternalInput").ap()
    out = nc.dram_tensor("out", [C, NH], F32, kind="ExternalOutput").ap()

    with tile.TileContext(nc) as tc:
        with tc.tile_pool(name="persist", bufs=1) as persist:
            rgbf = persist.tile([128, 2, NH], F32R, tag="rgbf", name="rgbf")
            ir8 = persist.tile([128, 2, N], FP8, tag="ir8", name="ir8")
            qsg = [persist.tile([128, 2, 512], FP8, tag=f"qsg{g}",
                                name=f"qsg{g}") for g in range(IG)]
            vTp = persist.tile([128, PAIRS, 2, 257], FP8, tag="vTp",
                               name="vTp")
            znb = [persist.tile([128, ITG, 256], BF16, tag=f"znb{g}",
                                name=f"znb{g}") for g in range(IG)]
            rvb = [persist.tile([128, ITG], F32, tag=f"rvb{g}",
                                name=f"rvb{g}") for g in range(IG)]
            wqk_sb = persist.tile([128, 2, 128], F32R, tag="wqk_sb",
                                  name="wqk_sb")
            bqk_sb = persist.tile([128, 2], F32, tag="bqk_sb", name="bqk_sb")
            wv_sb = persist.tile([128, 2, 256], U8, tag="wv_sb", name="wv_sb")
            b4_sb = persist.tile([128, 2], F32, tag="b4_sb", name="b4_sb")
            ident = persist.tile([128, 128], BF16, tag="ident", name="ident")

            for ch in range(2):
                nc.sync.dma_start(wqk_sb[:, ch], wqk[ch])
            nc.sync.dma_start(bqk_sb[:], bqk)
            nc.sync.dma_start(wv_sb[:], wvp)
            nc.sync.dma_start(b4_sb[:], b4c)
            make_identity(nc, ident[:])
            nc.vector.memset(vTp[:, :, :, 256], 1.0)

            # ir: DRAM f32 -> SBUF fp8 pair layout via gpsimd cast-DMA,
            # quarter-granular so consumers start early.  rgb via HWDGE.
            for quarter in range(4):
                nsl = slice(quarter * 1024, (quarter + 1) * 1024)
                for ch in range(2):
                    nc.gpsimd.dma_start(ir8[:, ch, nsl],
                                        x_ir[ch * 128:(ch + 1) * 128, nsl])
            for g in range(IG):
                gsl = slice(g * 512, (g + 1) * 512)
                for ch in range(2):
                    nc.sync.dma_start(
                        rgbf[:, ch, gsl],
                        x_rgb[ch * 128:(ch + 1) * 128, gsl])

            wv8 = wv_sb[:].bitcast(FP8)

            # ---------------- Phase 2: q' conv + v' conv ------------------
            with (
                tc.tile_pool(name="qp_psum", bufs=2, space="PSUM") as qp,
                tc.tile_pool(name="vp_psum", bufs=2, space="PSUM") as vp,
            ):
                def vconv(pr):
                    # one DoubleRow matmul per j-tile, pair-packed psum
                    ps = vp.tile([128, 2, 256], F32, tag="vps", name="vps")
                    for m in range(2):
                        jt = 2 * pr + m
                        jsl = slice(jt * 128, (jt + 1) * 128)
                        nc.tensor.matmul(ps[:, m], ir8[:, :, jsl], wv8,
                                         perf_mode=DR, start=True, stop=True)
                    dst = vTp[:, pr, :, 0:256]
                    if pr % 2 == 0:
                        nc.scalar.copy(dst, ps[:])
                    else:
                        nc.vector.tensor_copy(dst, ps[:])

                def qconv(g):
                    gsl = slice(g * 512, (g + 1) * 512)
                    for ch in range(2):
                        ps = qp.tile([128, 512], F32, tag="qps", name="qps")
                        nc.tensor.matmul(ps[:], wqk_sb[:, ch],
                                         rgbf[:, ch, gsl],
                                         start=True, stop=True)
                        if (g + ch) % 2 == 0:
                            nc.scalar.activation(qsg[g][:, ch], ps[:],
                                                 AF.Identity,
                                                 bias=bqk_sb[:, ch:ch + 1])
                        else:
                            nc.vector.tensor_scalar_add(
                                qsg[g][:, ch], ps[:], bqk_sb[:, ch:ch + 1])

                for pr in range(4):
                    vconv(pr)
                qconv(0)
                qconv(1)
                for pr in range(4, 10):
                    vconv(pr)
                qconv(2)
                qconv(3)
                for pr in range(10, PAIRS):
                    vconv(pr)

            # ---- Phase 3+5: fused attention, phase5 pipelined ------------
            with (
                tc.tile_pool(name="pexp", bufs=8) as pexp,
                tc.tile_pool(name="ph5", bufs=3) as ph5,
                tc.tile_pool(name="s_psum", bufs=2, space="PSUM") as sp,
                tc.tile_pool(name="z_psum", bufs=4, space="PSUM") as zp,
            ):
                def phase5_steps(g, zps):
                    """Deferred phase5 for group g as a list of closures."""
                    gsl = slice(g * 512, (g + 1) * 512)
                    state = {}

                    def s_norm():
                        rv = rvb[g]
                        for t in range(ITG):
                            nc.vector.reciprocal(rv[:, t:t + 1],
                                                 zps[t][:, 256:257])
                        nc.vector.tensor_scalar_mul(rv[:], rv[:], 1.0 / VSCALE)
                        for t in range(ITG):
                            if t % 2 == 0:
                                nc.scalar.activation(znb[g][:, t],
                                                     zps[t][:, 0:256],
                                                     AF.Copy,
                                                     scale=rv[:, t:t + 1])
                            else:
                                nc.vector.tensor_scalar_mul(
                                    znb[g][:, t], zps[t][:, 0:256],
                                    rv[:, t:t + 1])

                    def s_transpose():
                        yT = zp.tile([128, 2, 512], BF16, tag="zt", name="yT")
                        for t in range(ITG):
                            for ch in range(2):
                                nc.tensor.transpose(
                                    yT[:, ch, t * 128:(t + 1) * 128],
                                    znb[g][:, t, ch * 128:(ch + 1) * 128],
                                    ident[:])
                        state["yT"] = yT

                    def s_act():
                        yT = state["yT"]
                        u = ph5.tile([128, 2, 512], BF16, tag="u", name="u")
                        yb = ph5.tile([128, 2, 512], BF16, tag="yb",
                                      name="yb")
                        for ch in range(2):
                            nc.scalar.activation(u[:, ch], yT[:, ch], AF.Exp,
                                                 bias=b4_sb[:, ch:ch + 1])
                            nc.vector.tensor_scalar_add(
                                yb[:, ch], yT[:, ch], b4_sb[:, ch:ch + 1])
                        s = ph5.tile([128, 2, 512], BF16, tag="s", name="s")
                        nc.scalar.activation(s[:], u[:], AF.Square, bias=1.0)
                        state["yb"], state["s"] = yb, s

                    def s_tail():
                        yb, s = state["yb"], state["s"]
                        d = ph5.tile([128, 2, 512], BF16, tag="d", name="d")
                        nc.gpsimd.tensor_scalar_add(d[:], s[:], 1.0)
                        r = ph5.tile([128, 2, 512], BF16, tag="r", name="r")
                        with nc.allow_low_precision(
                                reason="bf16 mish tail; 2e-2 gate"):
                            nc.vector.reciprocal(r[:], d[:])
                        tt = ph5.tile([128, 2, 512], BF16, tag="tt",
                                      name="tt")
                        nc.vector.tensor_scalar(tt[:], r[:], -2.0, 1.0,
                                                ALU.mult, ALU.add)
                        m = ph5.tile([128, 2, 512], BF16, tag="m", name="m")
                        nc.gpsimd.tensor_mul(m[:], yb[:], tt[:])
                        o = ph5.tile([128, 2, 512], F32, tag="o", name="o")
                        nc.gpsimd.tensor_add(o[:], m[:],
                                             rgbf[:, :, gsl].bitcast(F32))
                        for ch in range(2):
                            nc.sync.dma_start(
                                out[ch * 128:(ch + 1) * 128, gsl], o[:, ch])

                    return [s_norm, s_transpose, s_act, s_tail]

                # checkpoints in the next group's pair loop where the
                # previous group's phase5 steps are emitted
                CKPT = {1: 0, 4: 1, 7: 2, 10: 3}

                prev_steps = None
                for g in range(IG):
                    zps = [zp.tile([128, 257], F32, tag="zt", name=f"z{t}")
                           for t in range(ITG)]
                    pending = []

                    def flush(pair, zps=zps):
                        pt, pr = pair
                        for t in range(ITG):
                            nc.tensor.matmul(
                                zps[t][:], pt[:, :, t * 128:(t + 1) * 128],
                                vTp[:, pr], perf_mode=DR,
                                start=(pr == 0), stop=(pr == PAIRS - 1))

                    for pr in range(PAIRS):
                        if prev_steps is not None and pr in CKPT:
                            prev_steps[CKPT[pr]]()
                        ps = sp.tile([128, 2, 512], F32, tag="sT", name="sT")
                        for m in range(2):
                            jt = 2 * pr + m
                            jsl = slice(jt * 128, (jt + 1) * 128)
                            nc.tensor.matmul(ps[:, m], ir8[:, :, jsl],
                                             qsg[g][:], perf_mode=DR,
                                             start=True, stop=True)
                        if len(pending) > ZLAG - 1:
                            flush(pending.pop(0))
                        pt = pexp.tile([128, 2, 512], FP8, tag="pt",
                                       name="pt")
                        if pr in DVE_PRS:
                            nc.vector.tensor_scalar(pt[:].bitcast(U8), ps[:],
                                                    EXP_A, EXP_B,
                                                    ALU.mult, ALU.add)
                        else:
                            nc.scalar.activation(pt[:], ps[:], AF.Exp,
                                                 scale=SCALE)
                        pending.append((pt, pr))
                    for pair in pending:
                        flush(pair)
                    prev_steps = phase5_steps(g, zps)

                for step in prev_steps:
                    step()

    nc.finalize()
    return nc


def _blockdiag_T(w, g0, g1):
    """lhsT chunk: [[w[g0].T, 0], [0, w[g1].T]] as [128, 128]."""
    m = np.zeros((128, 128), dtype=np.float64)
    m[:64, :64] = w[g0].T
    m[64:, 64:] = w[g1].T
    return m


def prep_inputs(rgb, ir, w_q, b_q, w_k, b_k, w_v, b_v, w4, b4,
                gamma, beta, rmean, rvar):
    """Host-side weight folding (weights-only; no data-dependent work)."""
    f64 = np.float64
    w_q, b_q = f64(np.asarray(w_q)), f64(np.asarray(b_q))
    w_k = f64(np.asarray(w_k))
    w_v, b_v = f64(np.asarray(w_v)), f64(np.asarray(b_v))
    w4, b4 = f64(np.asarray(w4)), f64(np.asarray(b4))
    gamma, beta = f64(np.asarray(gamma)), f64(np.asarray(beta))
    rmean, rvar = f64(np.asarray(rmean)), f64(np.asarray(rvar))

    inv = gamma / np.sqrt(rvar + 1e-5)
    w4f = w4 * inv[:, None]                      # BN folded into w4
    b4pp = b4 * inv + beta - rmean * inv + w4f @ b_v  # b_v folded (exact)

    # q'-side fold: scores = ir^T q',  q' = wqk rgb + bqk (block-diagonal)
    wqk = np.stack([w_k[g].T @ w_q[g] for g in range(G)])
    bqk = np.concatenate([w_k[g].T @ b_q[g * CG:(g + 1) * CG]
                          for g in range(G)])

    # v'-side fold: W' = w4f blockdiag(w_v), fp8-packed rhs [p, chunk, o]
    bd = np.zeros((C, C))
    for g in range(G):
        bd[g * CG:(g + 1) * CG, g * CG:(g + 1) * CG] = w_v[g]
    Wp = (w4f @ bd) * VSCALE
    WpT = Wp.T                                  # [a, o]
    wvp = np.ascontiguousarray(
        WpT.reshape(2, 128, C).transpose(1, 0, 2))
    wvp_u8 = wvp.astype(np.float32).astype(
        ml_dtypes.float8_e4m3).view(np.uint8)

    f32 = np.float32
    wqk_bd = np.stack([_blockdiag_T(wqk, 0, 1),
                       _blockdiag_T(wqk, 2, 3)]).astype(f32)

    def cols(v):
        return np.stack([v[:128], v[128:]], axis=1).astype(f32)

    bqk_c = cols(bqk)
    b4_c = cols(b4pp)

    rgb_f = np.ascontiguousarray(np.asarray(rgb), dtype=f32)
    ir_f = np.ascontiguousarray(np.asarray(ir), dtype=f32)

    weights = dict(wqk=wqk_bd, bqk=bqk_c, wvp=wvp_u8, b4c=b4_c)
    in_maps = []
    for core in range(NCORES):
        b, half = divmod(core, 2)
        x_rgb = np.ascontiguousarray(
            rgb_f[b].reshape(C, N)[:, half * NH:(half + 1) * NH])
        x_ir = np.ascontiguousarray(ir_f[b].reshape(C, N))
        in_maps.append(dict(x_rgb=x_rgb, x_ir=x_ir, **weights))
    return in_maps


_PROGRAM = None


def _get_program():
    global _PROGRAM
    if _PROGRAM is None:
        _PROGRAM = build_program()
    return _PROGRAM


def run(inputs, trace=False, **kw):
    """Run on 8 cores; returns (full_output, BassKernelResults)."""
    nc = _get_program()
    in_maps = prep_inputs(**inputs)
    res = run_bass_kernel_spmd(nc, in_maps, list(range(NCORES)),
                               trace=trace, **kw)
    full = np.zeros((BS, C, H, W), dtype=np.float32)
    for core in range(NCORES):
        b, half = divmod(core, 2)
        full[b].reshape(C, N)[:, half * NH:(half + 1) * NH] = \
            res.results[core]["out"]
    return full, res


def kernel(**inputs) -> np.ndarray:
    out, _ = run(inputs)
    return out
